# revision 9
# baseline (speedup 1.0000x reference)
"""DPConv (kernel=8, ext=4, stride=4) on 8 TRN2 NeuronCores.

Math: with K = k + 2e = 16 and k = 8, every adaptive-pool bin is exactly
2 wide, so the whole DPConv collapses to a separable linear operator:

    out_img = L @ img @ L.T          (per (n, c) image, 128x128)

where L is a 128x128 stencil matrix: for output index w the contributing
windows are i in [max(0,ceil((w-7)/4)), min(30, floor(w/4))] (counted
twice when that range is a single i - the fold count normalization),
each contributing the clamped replicate-padded pair {2w-4i-4, 2w-4i-3}
with weight 1/4 (pool avg 1/2 x fold avg 1/2).

The kernel is pure DMA-bound (target_regime=memory): 2 MiB in + 2 MiB
out per core.  Both directions ride bf16 (the rel-err budget is 2e-2;
bf16 end-to-end lands at ~5e-3), which halves HBM traffic vs the fp32
baseline.  L is exact in bf16 (entries are n/16), so a single full-rate
bf16 matmul does the row pass with fp32 PSUM accumulation.

On-chip factorization per image tile [H=128 partitions, W=128 free]:
  rows:  T = (L/4) @ x on TensorE (PSUM f32).  The HOST permutes each
         image's columns to [evens | odds], so T's free dim holds the
         even-w columns in 0..63 and odd-w in 64..127: the pairsum
         P[v] = T[2v-4] + T[2v-3] becomes an add of two CONTIGUOUS
         64-col blocks (PSUM has 8-byte cachelines - the natural
         stride-2 read would run at half rate).
  cols:  ACT evacuates T[0:64] PSUM->SBUF; DVE adds T[64:128] straight
         from PSUM, casting to bf16 (P).  Fold out[4a+b] = P[2a+b] +
         P[2a+b+2] runs bf16 in/out, split DVE (a=1..22) / GpSimd
         (a=23..30) - measured DVE bf16 folds are ~7x cheaper per col
         than GpSimd.  P-edge cols ride ACT, out-edge cols GpSimd.

Sharding: pure data parallel - core k takes batch element n = k.
Host staging per core: transpose to [H, C, W], permute W to
[evens | odds], cast bf16.  Output returns [H, C, W] bf16, upcast +
transposed on the host.

DMA schedule (all on the sync/SP HWDGE ring, in FIFO order):
  lt (32 KiB) -> loads [8, 24, 32] images -> stores [16, 24, 16, 8].
  The small leading load starts compute early; the larger tail loads
  keep per-partition descriptor runs long (6-8 KiB) for DMA
  efficiency; the small trailing store shrinks the drain tail.  Few
  DMAs and few tile buffers keep the semaphore count down - the
  framework's end-of-kernel per-semaphore reset storm is ~100 ns per
  semaphore on the critical path.
"""

import ml_dtypes
import numpy as np

import concourse.bacc as bacc
import concourse.mybir as mybir
import concourse.tile as tile
from concourse import bass_utils
from concourse.ap import AP

N_CORES = 8
C_PER_CORE = 64          # images per core (= C; one batch element per core)
G = 8                    # images per compute group (PSUM: [128,G,128] f32)
LOAD_CHUNKS = (8, 24, 32)            # images per input DMA
STORE_CHUNKS = (16, 24, 16, 8)       # images per output DMA
DVE_FOLD_A = 22          # fold a-slots on DVE (cols 4..4+4*A-1); rest GpSimd
N_GROUPS = C_PER_CORE // G
F32 = mybir.dt.float32
BF16 = mybir.dt.bfloat16
BF16_NP = ml_dtypes.bfloat16
assert sum(LOAD_CHUNKS) == C_PER_CORE and sum(STORE_CHUNKS) == C_PER_CORE

# host-side column permutation: [evens | odds]
_PERM = np.concatenate([np.arange(0, 128, 2), np.arange(1, 128, 2)])


def _build_lq() -> np.ndarray:
    """The 1-D DPConv operator with both 1/4 scalings folded in: L/4."""
    L = np.zeros((128, 128), np.float64)
    for w in range(128):
        i_lo = max(0, -((7 - w) // 4))      # ceil((w-7)/4)
        i_hi = min(30, w // 4)
        for i in (i_lo, i_hi):              # counted twice when equal
            L[w, min(127, max(0, 2 * w - 4 * i - 4))] += 0.25
            L[w, min(127, max(0, 2 * w - 4 * i - 3))] += 0.25
    return (L / 4.0).astype(np.float32)


_LQ_T = np.ascontiguousarray(_build_lq().T)          # lhsT layout [r, h]
_LQ_T_BF16 = _LQ_T.astype(BF16_NP)
assert np.all(_LQ_T_BF16.astype(np.float32) == _LQ_T)  # L exact in bf16


def _as_strided(base: AP, dims) -> AP:
    """Rebuild `base` (a sliced AP pointing at the wanted offset) with
    explicit [stride, size] free dims (overlapping reads allowed)."""
    return AP(base.tensor, base.offset, dims)


def _chunk_starts(chunks):
    s, out = 0, []
    for c in chunks:
        out.append(s)
        s += c
    return out


def _dpconv_tile(tc, o_d, xb_d, lt_d):
    nc = tc.nc
    load_starts = _chunk_starts(LOAD_CHUNKS)
    store_starts = _chunk_starts(STORE_CHUNKS)
    with tc.tile_pool(name="const", bufs=1) as cp, \
         tc.tile_pool(name="in", bufs=1) as inp, \
         tc.tile_pool(name="io", bufs=1) as iop, \
         tc.tile_pool(name="mid", bufs=3) as mp, \
         tc.tile_pool(name="ps", bufs=4, space="PSUM") as pp:
        # lt first on the ring: 32 KiB, delays the first load by ~200ns
        # but lets the first matmul fire as soon as load0 lands.
        lt = cp.tile([128, 128], BF16)
        nc.sync.dma_start(out=lt[:], in_=lt_d)

        # all input DMAs issued up-front on the sync/SP HWDGE ring;
        # every chunk has its own buffer so nothing back-pressures.
        in_tiles = {}                         # image index -> (tile, offset)
        for i, (c0, cn) in enumerate(zip(load_starts, LOAD_CHUNKS)):
            ct = inp.tile([128, cn, 128], BF16, tag=f"in{i}", name=f"ct{i}")
            nc.sync.dma_start(out=ct[:], in_=xb_d[:, c0:c0 + cn, :])
            for c in range(c0, c0 + cn):
                in_tiles[c] = (ct, c - c0)

        out_tiles = {}
        store_of_group = {}
        for i, (c0, cn) in enumerate(zip(store_starts, STORE_CHUNKS)):
            ot = iop.tile([128, cn, 128], BF16, tag=f"out{i}", name=f"ot{i}")
            for c in range(c0, c0 + cn):
                out_tiles[c] = (ot, c - c0)
            store_of_group[(c0 + cn) // G - 1] = (ot, c0, cn)

        for g in range(N_GROUPS):
            ct, cofs = in_tiles[g * G]

            # rows: T = (L/4) @ x; one bf16 matmul per 512-col PSUM bank
            t1 = pp.tile([128, G, 128], F32, tag="t1")
            for h in range(G // 4):
                cs = slice(4 * h, 4 * (h + 1))
                nc.tensor.matmul(t1[:, cs, :], lt[:],
                                 ct[:, cofs + 4 * h:cofs + 4 * (h + 1)],
                                 start=True, stop=True)

            # cols step 1: pairsum P[v] = T_e[v-2] + T_o[v-2] - the host
            # permuted image cols to [evens | odds], so both reads are
            # contiguous 64-col blocks.  TensorTensor may read at most
            # ONE input from PSUM: ACT evacuates the even block, DVE
            # adds the odd block straight from PSUM, casting to bf16.
            pe_t = mp.tile([128, G, 64], F32, tag="pe")
            nc.scalar.copy(out=pe_t[:], in_=t1[:, :, 0:64])
            pt = mp.tile([128, G, 68], BF16, tag="P")
            gdim = list(pt[:].ap[1])            # [68-ish pitch, G]
            pdim0 = list(pt[:].ap[0])           # partition dim
            tdim = t1[:].ap
            nc.vector.tensor_add(
                out=pt[:, :, 2:66], in0=pe_t[:], in1=t1[:, :, 64:128])
            # P edge cols {0,1,66,67} = 2x permuted-T cols {0,0,127,127}
            # (= original image cols {0,127}): one ACT op - out strides
            # (66,1), in strides (127, 0-broadcast)
            nc.scalar.mul(
                _as_strided(pt[:, :, 0:1], [pdim0, gdim, [66, 2], [1, 2]]),
                _as_strided(t1[:, :, 0:1],
                            [list(tdim[0]), list(tdim[1]), [127, 2], [0, 2]]),
                2.0)

            # cols step 2: fold out[4a+b] = P[2a+b] + P[2a+b+2], with
            # overlapping as-strided bf16 reads and contiguous bf16
            # writes, split DVE (a=1..DVE_FOLD_A) / GpSimd (rest).
            # Out-edge cols {0..3,124..127} = 2x P{0..3,64..67} ride
            # ACT as a scaled two-region copy.
            ot, oofs = out_tiles[g * G]
            odim = ot[:, oofs:oofs + G, :].ap
            na = DVE_FOLD_A
            nb = 30 - na
            in0a = _as_strided(pt[:, :, 2:3], [pdim0, gdim, [2, na], [1, 4]])
            in1a = _as_strided(pt[:, :, 4:5], [pdim0, gdim, [2, na], [1, 4]])
            out_a = _as_strided(
                ot[:, oofs:oofs + G, 4:5],
                [list(odim[0]), list(odim[1]), [4, na], [1, 4]])
            nc.vector.tensor_add(out=out_a, in0=in0a, in1=in1a)
            pb = 2 + 2 * na
            in0b = _as_strided(pt[:, :, pb:pb + 1],
                               [pdim0, gdim, [2, nb], [1, 4]])
            in1b = _as_strided(pt[:, :, pb + 2:pb + 3],
                               [pdim0, gdim, [2, nb], [1, 4]])
            out_b = _as_strided(
                ot[:, oofs:oofs + G, 4 + 4 * na:5 + 4 * na],
                [list(odim[0]), list(odim[1]), [4, nb], [1, 4]])
            nc.gpsimd.tensor_add(out=out_b, in0=in0b, in1=in1b)
            edge_in = _as_strided(pt[:, :, 0:1], [pdim0, gdim, [64, 2], [1, 4]])
            edge_out = _as_strided(
                ot[:, oofs:oofs + G, 0:1],
                [list(odim[0]), list(odim[1]), [124, 2], [1, 4]])
            nc.scalar.mul(edge_out, edge_in, 2.0)

            # stores ride the sync/SP ring behind the loads: SP has no
            # compute, so store sem-waits never head-of-line-block ACT
            # ops the way scalar-ring stores would.
            if g in store_of_group:
                ot, c0, cn = store_of_group[g]
                nc.sync.dma_start(out=o_d[:, c0:c0 + cn, :], in_=ot[:])


_CACHE = {}


def _get_nc():
    if "nc" not in _CACHE:
        nc = bacc.Bacc("TRN2", target_bir_lowering=False, debug=False)
        xb_d = nc.dram_tensor("xb", (128, C_PER_CORE, 128), BF16,
                              kind="ExternalInput").ap()
        lt_d = nc.dram_tensor("lt", (128, 128), BF16,
                              kind="ExternalInput").ap()
        o_d = nc.dram_tensor("o", (128, C_PER_CORE, 128), BF16,
                             kind="ExternalOutput").ap()
        with tile.TileContext(nc) as tc:
            _dpconv_tile(tc, o_d, xb_d, lt_d)
        nc.compile()
        _CACHE["nc"] = nc
    return _CACHE["nc"]


def _stage(xk: np.ndarray) -> np.ndarray:
    """[C,H,W] f32 -> [H,C,W] bf16, W permuted to [evens | odds] so the
    on-chip pairsum reads contiguous PSUM blocks."""
    return np.ascontiguousarray(
        xk.transpose(1, 0, 2)[:, :, _PERM]).astype(BF16_NP)


def run(x: np.ndarray, **spmd_kwargs) -> bass_utils.BassKernelResults:
    """Shard x (8,64,128,128) across 8 cores and run the Bass kernel."""
    nc = _get_nc()
    in_maps = [
        {"xb": _stage(x[k]), "lt": _LQ_T_BF16} for k in range(N_CORES)
    ]
    return bass_utils.run_bass_kernel_spmd(
        nc, in_maps, core_ids=list(range(N_CORES)), **spmd_kwargs)


def kernel(x) -> np.ndarray:
    x = np.asarray(x, dtype=np.float32)
    assert x.shape == (N_CORES, C_PER_CORE, 128, 128), x.shape
    res = run(x)
    return np.stack(
        [res.results[k]["o"].astype(np.float32).transpose(1, 0, 2)
         for k in range(N_CORES)],
        axis=0)
